# revision 1
# baseline (speedup 1.0000x reference)
"""CLUB loss kernel for Trainium2, 8-core data-parallel SPMD.

Math: with flat_x (N,D) [from x (B,D,H,W) -> (B*H*W, D)], v = exp(-p_logvar),
  loss = mean_i[ -0.5*sum_d ((x-mu)^2 - (m2 - 2*mu*m1 + mu^2)) * v ]
       = (-0.5/N) * [ A - 2B - dot(m2, V) + 2*dot(m1, W) ]
where
  A  = sum_{i,d} x^2 v          B  = sum_{i,d} x mu v
  V_d = sum_i v                 W_d = sum_i mu v
  m1 = S1/N, m2 = S2/N,  S1_d = sum_i x,  S2_d = sum_i x^2
All terms are per-core-local partial sums; the tiny (~KB) cross-core
reduction and final dot products happen on host in float64. No collectives.

Layout: d-major (partition = d) so every reduction above is a free-axis sum
riding on `accum_out` of ops we need anyway; PE does only 128x128 block
transposes of mu/logvar (identity matmuls into PSUM) and no reduction
matmuls. Engine split: ACT = exp (fused with lvT PSUM evacuation + V), x^2
(+S2), copy (+S1) — the x-only passes run early; DVE = the three product
passes w/a/b with their W/A/B reductions fused, kept per-512-wide half so
the post-last-DMA tail stays short.

Streaming: 512 KiB mu/lv slabs + 512 KiB x quarters issued interleaved so
complete (b-block, i-half, d-chunk) work becomes ready uniformly across the
~36 us DMA window and the engines pipeline directly behind the stream.
"""

import sys

import numpy as np

for _p in ("/opt/trn_rl_repo",):
    if _p not in sys.path:
        sys.path.append(_p)

B, D, H, W = 16, 512, 32, 32
HW = H * W
N = B * HW
NCORES = 8
BLKB = B // NCORES          # b-blocks per core (2)
ROWS = N // NCORES          # rows per core (2048)
NT = ROWS // 128            # 128-row i-tiles per core (16)
NDC = D // 128              # d chunks (4)
SLAB = 2                    # i-tiles per mu/lv DMA slab (= 256 i, 512 KiB)
NU = BLKB * NDC             # full units per core (8)
HHW = HW // 2               # i-extent of a half-unit (512)

_prog_cache = {}


def build_program():
    import concourse.bacc as bacc
    import concourse.tile as tile
    from concourse import mybir

    f32 = mybir.dt.float32
    AF = mybir.ActivationFunctionType
    OP = mybir.AluOpType

    nc = bacc.Bacc(
        "TRN2",
        target_bir_lowering=False,
        debug=False,
        enable_asserts=False,
        num_devices=NCORES,
    )

    x_d = nc.dram_tensor("x_s", (BLKB, D, HW), f32, kind="ExternalInput").ap()
    mu_d = nc.dram_tensor("mu_s", (ROWS, D), f32, kind="ExternalInput").ap()
    lv_d = nc.dram_tensor("lv_s", (ROWS, D), f32, kind="ExternalInput").ap()
    id_d = nc.dram_tensor("ident", (128, 128), f32, kind="ExternalInput").ap()

    # o_misc columns (partition p -> d = 128*dc+p), unit u = b*NDC+dc,
    # half-col hc = 2*u+h:
    #   [0,2NU) V | [2NU,4NU) W | [4NU,6NU) A | [6NU,8NU) B   (per half)
    #   [8NU,9NU) S1 | [9NU,10NU) S2                          (per unit)
    o_misc = nc.dram_tensor("o_misc", (128, 10 * NU), f32, kind="ExternalOutput").ap()

    with tile.TileContext(nc) as tc:
        with (
            tc.tile_pool(name="const", bufs=1) as constp,
            tc.tile_pool(name="xnat", bufs=1) as xp,
            tc.tile_pool(name="slab", bufs=8) as slp,
            tc.tile_pool(name="vw", bufs=6) as vwp,
            tc.tile_pool(name="ppool", bufs=8) as ppool,
            tc.tile_pool(name="scr", bufs=3) as scrp,
            tc.tile_pool(name="accum", bufs=1) as accp,
            tc.tile_pool(name="psum", bufs=4, space="PSUM") as pp,
        ):
            ident = constp.tile([128, 128], f32)

            acc = [
                accp.tile([128, w_], f32, tag=f"acc{q}", name=f"acc{q}")
                for q, w_ in enumerate((2 * NU, 2 * NU, 2 * NU, 2 * NU, NU, NU))
            ]

            lv_slabs = {}
            mu_slabs = {}
            xq = {}

            def load_slab(dram, store, sidx, tag, eng=None):
                rows = dram[128 * SLAB * sidx : 128 * SLAB * (sidx + 1), :]
                t_ = slp.tile([128, SLAB * D], f32, tag=tag, name=tag)
                (eng or nc.gpsimd).dma_start(
                    t_[:], rows.rearrange("(g p) f -> p g f", p=128)
                )
                store[sidx] = t_

            def load_x_quarter(b, dc):
                t_ = xp.tile([128, HW], f32, tag=f"x_{b}_{dc}", name=f"x_{b}_{dc}")
                # scalar(ACT) HWDGE queue: parallelizes issue with sync/gpsimd
                # during boot (ACT is idle until the first lvT lands anyway)
                nc.scalar.dma_start(t_[:], x_d[b, 128 * dc : 128 * (dc + 1), :])
                xq[(b, dc)] = t_

            def load_half_block(b, h, eng=None):
                # slabs covering i-tiles [8b+4h, 8b+4h+4) = 2 slabs per tensor
                s0 = (8 * b + 4 * h) // SLAB
                for s in (s0, s0 + 1):
                    load_slab(lv_d, lv_slabs, s, "lv_sl", eng)
                for s in (s0, s0 + 1):
                    load_slab(mu_d, mu_slabs, s, "mu_sl", eng)

            # interleaved issue order for uniform readiness; the very first
            # lv slab rides the Sync sequencer (earliest boot) ahead of the
            # identity and x loads so transposes can start sooner
            load_slab(lv_d, lv_slabs, 0, "lv_sl", eng=nc.sync)
            nc.sync.dma_start(ident[:], id_d[:])
            load_slab(lv_d, lv_slabs, 1, "lv_sl", eng=nc.sync)
            load_slab(mu_d, mu_slabs, 0, "mu_sl")
            load_slab(mu_d, mu_slabs, 1, "mu_sl")
            for dc in range(NDC):
                load_x_quarter(0, dc)
            load_half_block(0, 1)

            phold = {}
            for b in range(BLKB):
                for h in range(2):
                    if b > 0 and h == 0:
                        # x quarters ahead of the slabs: their x-only ACT
                        # passes must not queue behind slab-gated exps
                        for dc in range(NDC):
                            load_x_quarter(b, dc)
                        load_half_block(b, 0)
                    if b > 0 and h == 1:
                        load_half_block(b, 1)

                    for dc in range(NDC):
                        u = b * NDC + dc
                        hc = 2 * u + h
                        xs = xq[(b, dc)][:, HHW * h : HHW * (h + 1)]

                        if h == 0 and b > 0:
                            # later blocks: emit the x-only ACT passes FIRST
                            # so ACT's in-order queue can run them the moment
                            # x lands instead of stalling behind exp
                            xfull = xq[(b, dc)][:]
                            p_u = ppool.tile([128, HW], f32, tag="p", name="p_u")
                            phold[dc] = p_u
                            nc.scalar.activation(
                                p_u[:], xfull, AF.Square,
                                accum_out=acc[5][:, u : u + 1],
                            )
                            s1scr = scrp.tile(
                                [128, HW], f32, tag="s1scr", name="s1scr"
                            )
                            nc.scalar.activation(
                                s1scr[:], xfull, AF.Copy,
                                accum_out=acc[4][:, u : u + 1],
                            )

                        lvT = pp.tile([128, HHW], f32, tag="lvT", name="lvT")
                        muT = pp.tile([128, HHW], f32, tag="muT", name="muT")
                        for blk in range(4):
                            t_i = 8 * b + 4 * h + blk
                            sl_col = D * (t_i % SLAB) + 128 * dc
                            for dst, store in ((lvT, lv_slabs), (muT, mu_slabs)):
                                nc.tensor.matmul(
                                    dst[:, 128 * blk : 128 * (blk + 1)],
                                    store[t_i // SLAB][:, sl_col : sl_col + 128],
                                    ident[:],
                                    is_transpose=True,
                                    start=(blk == 0),
                                    stop=(blk == 3),
                                )

                        v_u = vwp.tile([128, HHW], f32, tag="v", name="v_u")
                        nc.scalar.activation(
                            v_u[:], lvT[:], AF.Exp, scale=-1.0,
                            accum_out=acc[0][:, hc : hc + 1],
                        )
                        w_u = vwp.tile([128, HHW], f32, tag="w", name="w_u")
                        nc.vector.scalar_tensor_tensor(
                            out=w_u[:], in0=muT[:], scalar=1.0, in1=v_u[:],
                            op0=OP.mult, op1=OP.mult,
                            accum_out=acc[1][:, hc : hc + 1],
                        )

                        if h == 0 and b == 0:
                            # first block: x lands after lv, keep exp first
                            xfull = xq[(b, dc)][:]
                            p_u = ppool.tile([128, HW], f32, tag="p", name="p_u")
                            phold[dc] = p_u
                            nc.scalar.activation(
                                p_u[:], xfull, AF.Square,
                                accum_out=acc[5][:, u : u + 1],
                            )
                            s1scr = scrp.tile(
                                [128, HW], f32, tag="s1scr", name="s1scr"
                            )
                            nc.scalar.activation(
                                s1scr[:], xfull, AF.Copy,
                                accum_out=acc[4][:, u : u + 1],
                            )

                        ph = phold[dc][:, HHW * h : HHW * (h + 1)]
                        a_scr = scrp.tile([128, HHW], f32, tag="a", name="a_scr")
                        nc.vector.scalar_tensor_tensor(
                            out=a_scr[:], in0=ph, scalar=1.0, in1=v_u[:],
                            op0=OP.mult, op1=OP.mult,
                            accum_out=acc[2][:, hc : hc + 1],
                        )
                        b_scr = scrp.tile([128, HHW], f32, tag="b", name="b_scr")
                        nc.vector.scalar_tensor_tensor(
                            out=b_scr[:], in0=w_u[:], scalar=1.0, in1=xs,
                            op0=OP.mult, op1=OP.mult,
                            accum_out=acc[3][:, hc : hc + 1],
                        )

            off = 0
            for q in (0, 1, 2, 3, 4, 5):
                w_ = acc[q].shape[1]
                nc.sync.dma_start(o_misc[:, off : off + w_], acc[q][:])
                off += w_

    nc.compile()
    return nc


def get_program():
    if "nc" not in _prog_cache:
        _prog_cache["nc"] = build_program()
    return _prog_cache["nc"]


def make_in_maps(x, p_mu, p_logvar):
    x = np.ascontiguousarray(np.asarray(x, dtype=np.float32)).reshape(B, D, HW)
    p_mu = np.ascontiguousarray(np.asarray(p_mu, dtype=np.float32))
    p_logvar = np.ascontiguousarray(np.asarray(p_logvar, dtype=np.float32))
    in_maps = []
    for c in range(NCORES):
        in_maps.append(
            {
                "x_s": np.ascontiguousarray(x[BLKB * c : BLKB * (c + 1)]),
                "mu_s": np.ascontiguousarray(p_mu[ROWS * c : ROWS * (c + 1)]),
                "lv_s": np.ascontiguousarray(p_logvar[ROWS * c : ROWS * (c + 1)]),
                "ident": np.eye(128, dtype=np.float32),
            }
        )
    return in_maps


def finish_host(results):
    """Combine per-core partials (float64) into the scalar loss."""
    Vv = np.zeros(D)
    Ww = np.zeros(D)
    S2 = np.zeros(D)
    S1 = np.zeros(D)
    A = 0.0
    Bb = 0.0
    for r in results:
        misc = r["o_misc"].astype(np.float64)
        for u in range(NU):
            b, dc = divmod(u, NDC)
            dsl = slice(128 * dc, 128 * (dc + 1))
            for h in range(2):
                hc = 2 * u + h
                Vv[dsl] += misc[:, hc]
                Ww[dsl] += misc[:, 2 * NU + hc]
                A += float(misc[:, 4 * NU + hc].sum())
                Bb += float(misc[:, 6 * NU + hc].sum())
            S1[dsl] += misc[:, 8 * NU + u]
            S2[dsl] += misc[:, 9 * NU + u]
    m1 = S1 / N
    m2 = S2 / N
    S = A - 2.0 * Bb - float(np.dot(m2, Vv)) + 2.0 * float(np.dot(m1, Ww))
    return np.float32(-0.5 / N * S)


def run_on_device(x, p_mu, p_logvar, trace=False, **kw):
    from concourse import bass_utils

    nc = get_program()
    in_maps = make_in_maps(x, p_mu, p_logvar)
    return bass_utils.run_bass_kernel_spmd(
        nc, in_maps, list(range(NCORES)), trace=trace, **kw
    )


def kernel(x, p_mu, p_logvar):
    res = run_on_device(x, p_mu, p_logvar)
    return finish_host(res.results)



# revision 7
# speedup vs baseline: 1.0035x; 1.0035x over previous
"""CLUB loss kernel for Trainium2, 8-core data-parallel SPMD (i-major, bf16).

Math: with flat_x (N,D) [from x (B,D,H,W) -> (B*H*W, D)], v = exp(-p_logvar),
  loss = (-0.5/N) * [ A - 2B - dot(m2, V) + 2*dot(m1, W) ]
where
  A = sum_{i,d} x^2 v      B = sum_{i,d} x mu v
  V_d = sum_i v            W_d = sum_i mu v
  S1_d = sum_i x           S2_d = sum_i x^2     m1 = S1/N, m2 = S2/N
Per-core partials; tiny cross-core reduction + final dots on host in f64.

Layout: i-major (partition = token i, free = d).  mu/lv stream in their
natural (N, D) layout -- no transposes.  Only x (natural d-major) is
PE-transposed (64 identity matmuls/core).  All per-d reductions
(V/W/S1/S2) are ones-vector matmuls on the otherwise idle PE,
PSUM-accumulated across the 16 i-tiles.

bf16: mu is cast fp32->bf16 during the SWDGE DMA; ACT emits v=exp(-lv) and
xTb=copy(xT) as bf16.  All three DVE product passes then run in the 2x
perf mode.  All reductions (accum_out, PSUM matmul accum) stay fp32; the
algebraic cancellation between positive and negative terms is exact in
the bf16-rounded values, so only the ~4e-3 input quantization perturbs
the result (measured ~1e-3 rel err, tolerance 2e-2).

Engines/tile (128 i x 512 d): PE 4 transposes + 4 ones-MMs; ACT exp+copy;
DVE xx=xTb*xTb, w=mu*v, a=xx*v (accum->A), b=w*xTb (accum->B).
DMA rings: sync+scalar (HWDGE) carry lv fp32 + x fp32; gpsimd (SWDGE)
carries mu with cast.  Issue order approximates by-need delivery.
"""

import sys

import numpy as np

for _p in ("/opt/trn_rl_repo",):
    if _p not in sys.path:
        sys.path.append(_p)

B, D, H, W = 16, 512, 32, 32
HW = H * W
N = B * HW
NCORES = 8
BLKB = B // NCORES          # b-blocks per core (2)
ROWS = N // NCORES          # rows per core (2048)
NT = ROWS // 128            # 128-row i-tiles per core (16)
NDC = D // 128              # d chunks (4)

_prog_cache = {}


def build_program():
    import concourse.bacc as bacc
    import concourse.tile as tile
    from concourse import mybir

    f32 = mybir.dt.float32
    bf16 = mybir.dt.bfloat16
    AF = mybir.ActivationFunctionType
    OP = mybir.AluOpType

    nc = bacc.Bacc(
        "TRN2",
        target_bir_lowering=False,
        debug=False,
        enable_asserts=False,
        num_devices=NCORES,
    )

    x_d = nc.dram_tensor("x_s", (BLKB, D, HW), f32, kind="ExternalInput").ap()
    mu_d = nc.dram_tensor("mu_s", (ROWS, D), f32, kind="ExternalInput").ap()
    lv_d = nc.dram_tensor("lv_s", (ROWS, D), f32, kind="ExternalInput").ap()
    id_d = nc.dram_tensor("ident", (128, 128), f32, kind="ExternalInput").ap()

    # outputs: o_vec rows = [V, W, S2, S1] (f32, d-vectors);
    # o_ab cols = [A partials (16) | B partials (16)] per (partition, tile)
    o_vec = nc.dram_tensor("o_vec", (1, 4 * D), f32, kind="ExternalOutput").ap()
    o_ab = nc.dram_tensor("o_ab", (128, 2 * NT), f32, kind="ExternalOutput").ap()

    with tile.TileContext(nc) as tc:
        with (
            tc.tile_pool(name="const", bufs=1) as constp,
            tc.tile_pool(name="xnat", bufs=1) as xp,
            tc.tile_pool(name="lvsl", bufs=3) as lvp,
            tc.tile_pool(name="musl", bufs=4) as mup,
            tc.tile_pool(name="elem", bufs=3) as ep,
            tc.tile_pool(name="scr", bufs=2) as scrp,
            tc.tile_pool(name="accum", bufs=1) as accp,
            tc.tile_pool(name="psum", bufs=3, space="PSUM") as pp,
            tc.tile_pool(name="psv", bufs=1, space="PSUM") as pvp,
        ):
            ident = constp.tile([128, 128], f32)
            ones = constp.tile([128, 1], bf16, tag="ones", name="ones")
            nc.vector.memset(ones[:], 1.0)

            accAB = accp.tile([128, 2 * NT], f32, tag="accab", name="accab")
            vws = [
                pvp.tile([1, D], f32, tag=f"vws{q}", name=f"vws{q}")
                for q in range(4)
            ]
            evac = accp.tile([1, 4 * D], f32, tag="evac", name="evac")

            lv_slabs = {}   # slab s covers i-tiles 2s, 2s+1 (fp32)
            mu_units = {}   # unit t covers i-tile t (bf16, cast during DMA)
            xq = {}         # (b, dc) -> natural x quarter (fp32)

            def load_lv(s, eng):
                t_ = lvp.tile([128, 2 * D], f32, tag="lv", name="lv_sl")
                rows = lv_d[256 * s : 256 * (s + 1), :]
                eng.dma_start(t_[:], rows.rearrange("(g p) f -> p g f", p=128))
                lv_slabs[s] = t_

            def load_mu(t):
                t_ = mup.tile([128, D], bf16, tag="mu", name="mu_un")
                rows = mu_d[128 * t : 128 * (t + 1), :]
                nc.gpsimd.dma_start(t_[:], rows)
                mu_units[t] = t_

            def load_x(b, dc, eng):
                t_ = xp.tile([128, HW], f32, tag=f"x_{b}_{dc}", name=f"x{b}{dc}")
                eng.dma_start(t_[:], x_d[b, 128 * dc : 128 * (dc + 1), :])
                xq[(b, dc)] = t_

            # ---- DMA issue (per-ring FIFO approximates by-need order) ----
            load_lv(0, nc.sync)
            nc.sync.dma_start(ident[:], id_d[:])
            load_x(0, 0, nc.scalar)
            load_x(0, 1, nc.sync)
            load_x(0, 2, nc.scalar)
            load_x(0, 3, nc.sync)
            load_lv(1, nc.scalar)
            load_lv(2, nc.sync)
            load_lv(3, nc.scalar)
            load_x(1, 0, nc.scalar)
            load_x(1, 1, nc.sync)
            load_x(1, 2, nc.scalar)
            load_x(1, 3, nc.sync)
            load_lv(4, nc.scalar)
            load_lv(5, nc.sync)
            load_lv(6, nc.scalar)
            load_lv(7, nc.sync)
            for t in range(NT):
                load_mu(t)

            # ---- compute: one (128 i, 512 d) tile per t ----
            for t in range(NT):
                b, j = divmod(t, 8)
                first, last = (t == 0), (t == NT - 1)

                # PE: xT = transpose of x(:, i-slice) -- 4 d-chunk blocks
                xT = pp.tile([128, D], f32, tag="xT", name="xT")
                for dc in range(NDC):
                    nc.tensor.matmul(
                        xT[:, 128 * dc : 128 * (dc + 1)],
                        xq[(b, dc)][:, 128 * j : 128 * (j + 1)],
                        ident[:],
                        is_transpose=True,
                        start=(dc == 0),
                        stop=(dc == NDC - 1),
                    )

                # ACT: v = exp(-lv) (bf16), then xTb = copy(xT) (bf16)
                lvsl = lv_slabs[t // 2][:, D * (t % 2) : D * (t % 2 + 1)]
                v_t = ep.tile([128, D], bf16, tag="v", name="v_t")
                nc.scalar.activation(v_t[:], lvsl, AF.Exp, scale=-1.0)
                xTb = ep.tile([128, D], bf16, tag="xTb", name="xTb")
                nc.scalar.activation(xTb[:], xT[:], AF.Copy)

                # DVE: products (all bf16 SBUF -> 2x mode)
                w_t = ep.tile([128, D], bf16, tag="w", name="w_t")
                nc.vector.scalar_tensor_tensor(
                    out=w_t[:], in0=mu_units[t][:], scalar=1.0, in1=v_t[:],
                    op0=OP.mult, op1=OP.mult,
                )
                xx = ep.tile([128, D], bf16, tag="xx", name="xx")
                nc.vector.scalar_tensor_tensor(
                    out=xx[:], in0=xTb[:], scalar=1.0, in1=xTb[:],
                    op0=OP.mult, op1=OP.mult,
                )
                a_scr = scrp.tile([128, D], bf16, tag="a", name="a_scr")
                nc.vector.scalar_tensor_tensor(
                    out=a_scr[:], in0=xx[:], scalar=1.0, in1=v_t[:],
                    op0=OP.mult, op1=OP.mult,
                    accum_out=accAB[:, t : t + 1],
                )
                b_scr = scrp.tile([128, D], bf16, tag="b", name="b_scr")
                nc.vector.scalar_tensor_tensor(
                    out=b_scr[:], in0=w_t[:], scalar=1.0, in1=xTb[:],
                    op0=OP.mult, op1=OP.mult,
                    accum_out=accAB[:, NT + t : NT + t + 1],
                )

                # PE: ones-matmuls accumulate V/W/S2/S1 across tiles
                for q, rhs in enumerate((v_t, w_t, xx, xTb)):
                    nc.tensor.matmul(
                        vws[q][:], ones[:], rhs[:],
                        start=first, stop=last,
                    )

            # evacuate PSUM d-vectors (ACT x2 + DVE x2, in parallel)
            nc.scalar.activation(evac[:, 0 * D : 1 * D], vws[0][:], AF.Copy)
            nc.vector.tensor_copy(evac[:, 1 * D : 2 * D], vws[1][:])
            nc.scalar.activation(evac[:, 2 * D : 3 * D], vws[2][:], AF.Copy)
            nc.vector.tensor_copy(evac[:, 3 * D : 4 * D], vws[3][:])

            nc.sync.dma_start(o_vec[:, :], evac[:])
            nc.sync.dma_start(o_ab[:, :], accAB[:])

    nc.compile()
    return nc


def get_program():
    if "nc" not in _prog_cache:
        _prog_cache["nc"] = build_program()
    return _prog_cache["nc"]


def make_in_maps(x, p_mu, p_logvar):
    x = np.ascontiguousarray(np.asarray(x, dtype=np.float32)).reshape(B, D, HW)
    p_mu = np.ascontiguousarray(np.asarray(p_mu, dtype=np.float32))
    p_logvar = np.ascontiguousarray(np.asarray(p_logvar, dtype=np.float32))
    ident = np.eye(128, dtype=np.float32)
    in_maps = []
    for c in range(NCORES):
        in_maps.append(
            {
                "x_s": np.ascontiguousarray(x[BLKB * c : BLKB * (c + 1)]),
                "mu_s": np.ascontiguousarray(p_mu[ROWS * c : ROWS * (c + 1)]),
                "lv_s": np.ascontiguousarray(p_logvar[ROWS * c : ROWS * (c + 1)]),
                "ident": ident,
            }
        )
    return in_maps


def finish_host(results):
    """Combine per-core partials (float64) into the scalar loss."""
    Vv = np.zeros(D)
    Ww = np.zeros(D)
    S2 = np.zeros(D)
    S1 = np.zeros(D)
    A = 0.0
    Bb = 0.0
    for r in results:
        vec = r["o_vec"].astype(np.float64).reshape(4, D)
        ab = r["o_ab"].astype(np.float64)
        Vv += vec[0]
        Ww += vec[1]
        S2 += vec[2]
        S1 += vec[3]
        A += float(ab[:, :NT].sum())
        Bb += float(ab[:, NT:].sum())
    m1 = S1 / N
    m2 = S2 / N
    S = A - 2.0 * Bb - float(np.dot(m2, Vv)) + 2.0 * float(np.dot(m1, Ww))
    return np.float32(-0.5 / N * S)


def run_on_device(x, p_mu, p_logvar, trace=False, **kw):
    from concourse import bass_utils

    nc = get_program()
    in_maps = make_in_maps(x, p_mu, p_logvar)
    return bass_utils.run_bass_kernel_spmd(
        nc, in_maps, list(range(NCORES)), trace=trace, **kw
    )


def kernel(x, p_mu, p_logvar):
    res = run_on_device(x, p_mu, p_logvar)
    return finish_host(res.results)


# revision 9
# speedup vs baseline: 1.2128x; 1.2085x over previous
"""CLUB loss kernel for Trainium2, 8-core data-parallel SPMD (i-major, fp16).

Math: with flat_x (N,D) [from x (B,D,H,W) -> (B*H*W, D)], v = exp(-p_logvar),
  loss = (-0.5/N) * [ A - 2B - dot(m2, V) + 2*dot(m1, W) ]
where
  A = sum_{i,d} x^2 v      B = sum_{i,d} x mu v
  V_d = sum_i v            W_d = sum_i mu v
  S1_d = sum_i x           S2_d = sum_i x^2     m1 = S1/N, m2 = S2/N
Per-core partials; tiny cross-core reduction + final dots on host in f64.

Layout: i-major (partition = token i, free = d).  mu/lv stream in their
natural (N, D) layout -- no transposes.  Only x (natural d-major) is
PE-transposed (64 fp16 identity matmuls/core, ~105ns each).  ALL six
reductions (V/W/S2/S1/Avec/Bvec, each a per-d column sum over i) are
ones-vector matmuls on the PE, PSUM-accumulated across the 16 i-tiles;
A/B are host sums of Avec/Bvec.

fp16 (not bf16): 8x finer mantissa at identical DVE/PE speed; all
intermediates (v<150, w<1k, a,b<6k) fit fp16 range.  HW-measured op
costs that drive this structure: DVE tensor_tensor f16 = 426ns (2x mode;
scalar_tensor_tensor is stuck at 1x/690ns, tensor_tensor_reduce crashes
the runtime), ACT activation = (N+352)/1.2 any dtype, fp16 transposes
~105ns effective, ones-MM ~220-430ns.

Engines/tile (128 i x 512 d): PE 4 transposes + 6 ones-chain MMs;
ACT exp (v, f16) + square (xx from PSUM xT, f16); DVE copy xTb (f16,
from PSUM) + w=mu*v + a=xx*v + b=w*xTb (all tensor_tensor f16 2x).

DMA: x and mu need fp32->fp16 casts so they ride the SWDGE (gpsimd)
ring, ordered by need [xh(group), mu, mu, ...]; lv (fp32, no cast) on
the two HWDGE rings (sync/scalar), throttled to consumption rate by
lv pool recycling (bufs=3) so the SWDGE ring gets the early bandwidth.
"""

import sys

import numpy as np

for _p in ("/opt/trn_rl_repo",):
    if _p not in sys.path:
        sys.path.append(_p)

B, D, H, W = 16, 512, 32, 32
HW = H * W
N = B * HW
NCORES = 8
BLKB = B // NCORES          # b-blocks per core (2)
ROWS = N // NCORES          # rows per core (2048)
NT = ROWS // 128            # 128-row i-tiles per core (16)
NDC = D // 128              # d chunks (4)
NG = 4                      # x groups per core (b-block halves), 4 tiles each

_prog_cache = {}


def build_program():
    import concourse.bacc as bacc
    import concourse.tile as tile
    from concourse import mybir

    f32 = mybir.dt.float32
    f16 = mybir.dt.float16
    AF = mybir.ActivationFunctionType
    OP = mybir.AluOpType

    nc = bacc.Bacc(
        "TRN2",
        target_bir_lowering=False,
        debug=False,
        enable_asserts=False,
        num_devices=NCORES,
    )

    x_d = nc.dram_tensor("x_s", (BLKB, D, HW), f32, kind="ExternalInput").ap()
    mu_d = nc.dram_tensor("mu_s", (ROWS, D), f32, kind="ExternalInput").ap()
    lv_d = nc.dram_tensor("lv_s", (ROWS, D), f32, kind="ExternalInput").ap()
    id_d = nc.dram_tensor("identh", (128, 128), f16, kind="ExternalInput").ap()

    # o_vec rows (after host reshape to (6, D)): [V, W, S2, S1, Avec, Bvec]
    o_vec = nc.dram_tensor("o_vec", (1, 6 * D), f32, kind="ExternalOutput").ap()

    with tile.TileContext(nc) as tc:
        with (
            tc.tile_pool(name="const", bufs=1) as constp,
            tc.tile_pool(name="xh", bufs=3) as xp,
            tc.tile_pool(name="lvsl", bufs=3) as lvp,
            tc.tile_pool(name="musl", bufs=3) as mup,
            tc.tile_pool(name="elem", bufs=3) as ep,
            tc.tile_pool(name="accum", bufs=1) as accp,
            tc.tile_pool(name="psum", bufs=2, space="PSUM") as pp,
            tc.tile_pool(name="psv", bufs=1, space="PSUM") as pvp,
        ):
            identh = constp.tile([128, 128], f16, tag="idh", name="idh")
            ones = constp.tile([128, 1], f16, tag="ones", name="ones")
            nc.vector.memset(ones[:], 1.0)

            vws = [
                pvp.tile([1, D], f32, tag=f"vws{q}", name=f"vws{q}")
                for q in range(6)
            ]
            evac = accp.tile([1, 6 * D], f32, tag="evac", name="evac")

            lv_slabs = {}   # slab s: i-tiles 2s, 2s+1 (fp32, natural)
            mu_slabs = {}   # slab s: i-tiles 2s, 2s+1 (f16 cast, natural)
            xhalf = {}      # group g: (128, NDC*512) f16, d-major half-block

            def load_lv(s, eng):
                t_ = lvp.tile([128, 2 * D], f32, tag="lv", name="lv_sl")
                rows = lv_d[256 * s : 256 * (s + 1), :]
                eng.dma_start(t_[:], rows.rearrange("(g p) f -> p g f", p=128))
                lv_slabs[s] = t_

            def load_mu(s):
                t_ = mup.tile([128, 2 * D], f16, tag="mu", name="mu_sl")
                rows = mu_d[256 * s : 256 * (s + 1), :]
                nc.gpsimd.dma_start(t_[:], rows.rearrange("(g p) f -> p g f", p=128))
                mu_slabs[s] = t_

            def load_xh(g):
                b, h = divmod(g, 2)
                t_ = xp.tile([128, NDC * 512], f16, tag="xh", name="xh")
                src = x_d[b, :, 512 * h : 512 * (h + 1)]
                nc.gpsimd.dma_start(t_[:], src.rearrange("(g p) f -> p g f", p=128))
                xhalf[g] = t_

            # ---- DMA issue ----
            # SWDGE ring (casts): by-need order; HW rings: lv, throttled
            # by lvp recycling so they don't front-run the SWDGE stream.
            nc.sync.dma_start(identh[:], id_d[:])
            for g in range(NG):
                load_xh(g)
                load_mu(2 * g)
                load_mu(2 * g + 1)
            for s in range(8):
                load_lv(s, nc.sync if s % 2 == 0 else nc.scalar)

            # ---- compute ----
            def transposes(t):
                g, jj = divmod(t, 4)
                xT = pp.tile([128, D], f16, tag="xT", name="xT")
                for dc in range(NDC):
                    col = 512 * dc + 128 * jj
                    nc.tensor.matmul(
                        xT[:, 128 * dc : 128 * (dc + 1)],
                        xhalf[g][:, col : col + 128],
                        identh[:],
                        is_transpose=True,
                        start=(dc == 0),
                        stop=(dc == NDC - 1),
                    )
                return xT

            xT_t = transposes(0)
            for t in range(NT):
                first, last = (t == 0), (t == NT - 1)
                xT = xT_t

                # ACT: v = exp(-lv) f16; xx = (xT)^2 f16 (evacuates PSUM)
                lvsl = lv_slabs[t // 2][:, D * (t % 2) : D * (t % 2 + 1)]
                v_t = ep.tile([128, D], f16, tag="v", name="v_t")
                nc.scalar.activation(v_t[:], lvsl, AF.Exp, scale=-1.0)
                xx = ep.tile([128, D], f16, tag="xx", name="xx")
                nc.scalar.activation(xx[:], xT[:], AF.Square)

                # DVE: xTb = copy(xT) f16; products (tensor_tensor, 2x)
                musl = mu_slabs[t // 2][:, D * (t % 2) : D * (t % 2 + 1)]
                w_t = ep.tile([128, D], f16, tag="w", name="w_t")
                nc.vector.tensor_tensor(w_t[:], musl, v_t[:], OP.mult)
                xTb = ep.tile([128, D], f16, tag="xTb", name="xTb")
                nc.vector.tensor_copy(xTb[:], xT[:])
                a_t = ep.tile([128, D], f16, tag="a", name="a_t")
                nc.vector.tensor_tensor(a_t[:], xx[:], v_t[:], OP.mult)
                b_t = ep.tile([128, D], f16, tag="b", name="b_t")
                nc.vector.tensor_tensor(b_t[:], w_t[:], xTb[:], OP.mult)

                # PE: prefetch next tile's transposes ahead of the chains
                if not last:
                    xT_t = transposes(t + 1)

                # PE: ones-chains accumulate V/W/S2/S1/Avec/Bvec
                for q, rhs in enumerate((v_t, w_t, xx, xTb, a_t, b_t)):
                    nc.tensor.matmul(
                        vws[q][:], ones[:], rhs[:],
                        start=first, stop=last,
                    )

            # evacuate PSUM d-vectors (3 on ACT, 3 on DVE, interleaved)
            for q in range(6):
                dst = evac[:, q * D : (q + 1) * D]
                if q % 2 == 0:
                    nc.scalar.activation(dst, vws[q][:], AF.Copy)
                else:
                    nc.vector.tensor_copy(dst, vws[q][:])

            nc.sync.dma_start(o_vec[:, :], evac[:])

    nc.compile()
    return nc


def get_program():
    if "nc" not in _prog_cache:
        _prog_cache["nc"] = build_program()
    return _prog_cache["nc"]


def make_in_maps(x, p_mu, p_logvar):
    x = np.ascontiguousarray(np.asarray(x, dtype=np.float32)).reshape(B, D, HW)
    p_mu = np.ascontiguousarray(np.asarray(p_mu, dtype=np.float32))
    p_logvar = np.ascontiguousarray(np.asarray(p_logvar, dtype=np.float32))
    identh = np.eye(128, dtype=np.float16)
    in_maps = []
    for c in range(NCORES):
        in_maps.append(
            {
                "x_s": np.ascontiguousarray(x[BLKB * c : BLKB * (c + 1)]),
                "mu_s": np.ascontiguousarray(p_mu[ROWS * c : ROWS * (c + 1)]),
                "lv_s": np.ascontiguousarray(p_logvar[ROWS * c : ROWS * (c + 1)]),
                "identh": identh,
            }
        )
    return in_maps


def finish_host(results):
    """Combine per-core partials (float64) into the scalar loss."""
    Vv = np.zeros(D)
    Ww = np.zeros(D)
    S2 = np.zeros(D)
    S1 = np.zeros(D)
    A = 0.0
    Bb = 0.0
    for r in results:
        vec = r["o_vec"].astype(np.float64).reshape(6, D)
        Vv += vec[0]
        Ww += vec[1]
        S2 += vec[2]
        S1 += vec[3]
        A += float(vec[4].sum())
        Bb += float(vec[5].sum())
    m1 = S1 / N
    m2 = S2 / N
    S = A - 2.0 * Bb - float(np.dot(m2, Vv)) + 2.0 * float(np.dot(m1, Ww))
    return np.float32(-0.5 / N * S)


def run_on_device(x, p_mu, p_logvar, trace=False, **kw):
    from concourse import bass_utils

    nc = get_program()
    in_maps = make_in_maps(x, p_mu, p_logvar)
    return bass_utils.run_bass_kernel_spmd(
        nc, in_maps, list(range(NCORES)), trace=trace, **kw
    )


def kernel(x, p_mu, p_logvar):
    res = run_on_device(x, p_mu, p_logvar)
    return finish_host(res.results)
